# revision 1
# baseline (speedup 1.0000x reference)
"""BlockwiseKronLinear forward on 8 trn2 NeuronCores.

Math: w = reshape(einsum('rij,rkl->ikjl', s*a, b), (4096, 64));
      out = x @ w + bias    with x (32768, 4096) fp32.

Strategy (data-parallel, per the batch axis):
  - Host: build the tiny w (1 MB) from the Kron factors, shard x along
    batch into 8 x 4096 rows, and hand each core its shard TRANSPOSED
    (xT: [4096 d, 4096 batch]) so the contraction dim lands on SBUF
    partitions without any on-device transpose (fp32 has no DMA-transpose
    path on trn2).
  - Device (identical SPMD program per core): stream xT in [128, 512]
    tiles, accumulate outT = w.T @ xT in PSUM over the 32 k-subtiles,
    fuse the bias add on ScalarE, write outT [64, 4096] back.
  - Host: gather, transpose back to [32768, 64].

The kernel is memory-bound: 64 MB of x per core vs ~360 GB/s HBM
=> ~178 us floor; PE work is ~32 matmul-streams of 512 cols each.
"""

import os
import sys

for _p in ("/opt/trn_rl_repo", "/root/.axon_site/_ro/trn_rl_repo"):
    if os.path.isdir(_p) and _p not in sys.path:
        sys.path.append(_p)

import numpy as np
from contextlib import ExitStack

import concourse.bass as bass
import concourse.tile as tile
from concourse import bacc, mybir
from concourse.bass_utils import run_bass_kernel_spmd

N_CORES = 8
BATCH, D, N = 32768, 4096, 64
RANK, A1, A2, B1, B2 = 16, 64, 8, 64, 8
SHARD = BATCH // N_CORES          # 4096 batch rows per core
P = 128                           # SBUF partitions
KSUB = D // P                     # 32 contraction subtiles
NB = 512                          # moving (batch) columns per matmul
NCHUNK = SHARD // NB              # 8 chunks per core

# 'fp32'  : exact fp32 matmul (PE runs it as 2 half-speed passes, 4 cyc/row)
# 'f32r'  : fp32 data, fast PE mode (1 cyc/row at N>=256); reduced precision
MM_MODE = os.environ.get("KRON_MM_MODE", "f32r")

_compiled = {}


def _build(mm_mode: str):
    if mm_mode in _compiled:
        return _compiled[mm_mode]

    nc = bacc.Bacc(
        "TRN2",
        target_bir_lowering=False,
        debug=False,
        num_devices=N_CORES,
    )
    xT = nc.dram_tensor("xT", [D, SHARD], mybir.dt.float32, kind="ExternalInput").ap()
    w = nc.dram_tensor("w", [D, N], mybir.dt.float32, kind="ExternalInput").ap()
    bias = nc.dram_tensor("bias", [N], mybir.dt.float32, kind="ExternalInput").ap()
    outT = nc.dram_tensor("outT", [N, SHARD], mybir.dt.float32, kind="ExternalOutput").ap()

    mm_dt = mybir.dt.float32r if mm_mode == "f32r" else mybir.dt.float32

    with tile.TileContext(nc) as tc, ExitStack() as ctx:
        const = ctx.enter_context(tc.tile_pool(name="const", bufs=1))
        xpool = ctx.enter_context(tc.tile_pool(name="x", bufs=2))
        opool = ctx.enter_context(tc.tile_pool(name="o", bufs=4))
        psum = ctx.enter_context(tc.tile_pool(name="psum", bufs=4, space="PSUM"))

        # w as [p, ksub, n]: d = ksub*128 + p
        w_sb = const.tile([P, KSUB, N], mybir.dt.float32)
        nc.sync.dma_start(w_sb[:], w.rearrange("(t p) n -> p t n", p=P))
        # bias along partitions (the output-feature dim of outT)
        bias_sb = const.tile([N, 1], mybir.dt.float32)
        nc.sync.dma_start(bias_sb[:], bias[:, None])

        xT_t = xT.rearrange("(t p) b -> p t b", p=P)
        for c in range(NCHUNK):
            x_sb = xpool.tile([P, KSUB, NB], mybir.dt.float32)
            for t in range(KSUB):
                nc.sync.dma_start(
                    x_sb[:, t], xT_t[:, t, c * NB : (c + 1) * NB]
                )
            ps = psum.tile([N, NB], mybir.dt.float32)
            for t in range(KSUB):
                nc.tensor.matmul(
                    ps[:],
                    lhsT=w_sb[:, t].bitcast(mm_dt),
                    rhs=x_sb[:, t].bitcast(mm_dt),
                    start=(t == 0),
                    stop=(t == KSUB - 1),
                )
            o_sb = opool.tile([N, NB], mybir.dt.float32)
            nc.scalar.activation(
                o_sb[:], ps[:], mybir.ActivationFunctionType.Identity,
                bias=bias_sb[:],
            )
            nc.sync.dma_start(outT[:, c * NB : (c + 1) * NB], o_sb[:])

    nc.compile()
    _compiled[mm_mode] = nc
    return nc


def _host_prep(x, s, a, b):
    sa = s[None, :, :].astype(np.float32) * a.astype(np.float32)
    w = np.einsum("rij,rkl->ikjl", sa, b.astype(np.float32))
    w = np.ascontiguousarray(w.reshape(D, N), dtype=np.float32)
    shards = [
        np.ascontiguousarray(x[i * SHARD : (i + 1) * SHARD].T)
        for i in range(N_CORES)
    ]
    return w, shards


def kernel(x, s, a, b, bias, _trace=False):
    w, shards = _host_prep(
        np.asarray(x, dtype=np.float32),
        np.asarray(s, dtype=np.float32),
        np.asarray(a, dtype=np.float32),
        np.asarray(b, dtype=np.float32),
    )
    bias = np.ascontiguousarray(np.asarray(bias, dtype=np.float32))
    nc = _build(MM_MODE)
    in_maps = [{"xT": shards[i], "w": w, "bias": bias} for i in range(N_CORES)]
    res = run_bass_kernel_spmd(nc, in_maps, list(range(N_CORES)), trace=_trace)
    out = np.concatenate(
        [np.asarray(r["outT"]).T for r in res.results], axis=0
    ).astype(np.float32)
    if _trace:
        return out, res
    return out


# revision 5
# speedup vs baseline: 1.0597x; 1.0597x over previous
"""BlockwiseKronLinear forward on 8 trn2 NeuronCores.

Math: w = reshape(einsum('rij,rkl->ikjl', s*a, b), (4096, 64));
      out = x @ w + bias    with x (32768, 4096) fp32.

Strategy (data-parallel along batch, per the sharding hint):
  - Host: build the tiny w (1 MB) from the Kron factors; shard x along
    batch into 8 x 4096 rows; lay each core's shard out TRANSPOSED and
    tiled ([p=128, chunk=8, ksub=32, nb=512], d = ksub*128 + p,
    batch = chunk*512 + nb) so the contraction dim lands on SBUF
    partitions (fp32 has no DMA-transpose on trn2) and every chunk is
    one fully-contiguous-per-partition 8 MB DMA.
  - Device (identical SPMD program per core): stream the chunks in,
    accumulate outT = w.T @ xT in PSUM over the contraction subtiles
    (w stationary, x the 512-wide moving operand), fuse the bias add on
    ScalarE, write outT [64, 4096] back.
  - Host: gather, transpose back to [32768, 64].

Matmul dtype modes (KRON_MM_MODE):
  - 'bf16x3' (default): split x and w into bf16 hi+lo on host and
    accumulate xh@wh + xl@wh + xh@wl in fp32 PSUM. 3 PE passes at
    1 cyc/row instead of fp32's 4 cyc/row; rel err ~1e-5.
  - 'fp32': exact fp32 matmul (PE runs it as 2x2 half-speed passes).

The kernel is memory-bound: 64 MB of x per core vs ~358 GB/s HBM
=> ~178 us floor.
"""

import os
import sys

for _p in ("/opt/trn_rl_repo", "/root/.axon_site/_ro/trn_rl_repo"):
    if os.path.isdir(_p) and _p not in sys.path:
        sys.path.append(_p)

import numpy as np
import ml_dtypes
from contextlib import ExitStack

import concourse.bass as bass
import concourse.tile as tile
from concourse import bacc, mybir
from concourse.bass_utils import run_bass_kernel_spmd

N_CORES = 8
BATCH, D, N = 32768, 4096, 64
SHARD = BATCH // N_CORES          # 4096 batch rows per core
P = 128                           # SBUF partitions
KSUB = D // P                     # 32 contraction subtiles
NB = 512                          # moving (batch) columns per matmul
NCHUNK = SHARD // NB              # 8 chunks per core

MM_MODE = os.environ.get("KRON_MM_MODE", "bf16x3")

_compiled = {}


def _build(mm_mode: str):
    if mm_mode in _compiled:
        return _compiled[mm_mode]

    nc = bacc.Bacc(
        "TRN2",
        target_bir_lowering=False,
        debug=False,
        num_devices=N_CORES,
    )
    f32 = mybir.dt.float32
    bf16 = mybir.dt.bfloat16

    bias = nc.dram_tensor("bias", [N], f32, kind="ExternalInput").ap()
    outT = nc.dram_tensor("outT", [N, SHARD], f32, kind="ExternalOutput").ap()

    if mm_mode == "bf16x3":
        # (x dram tensor, w dram tensor) per accumulation group
        xh = nc.dram_tensor("xh", [P, NCHUNK, KSUB, NB], bf16, kind="ExternalInput").ap()
        xl = nc.dram_tensor("xl", [P, NCHUNK, KSUB, NB], bf16, kind="ExternalInput").ap()
        wh = nc.dram_tensor("wh", [P, KSUB, N], bf16, kind="ExternalInput").ap()
        wl = nc.dram_tensor("wl", [P, KSUB, N], bf16, kind="ExternalInput").ap()
        x_drams, w_drams, mm_dt = [xh, xl], [wh, wl], bf16
        # (x_idx, w_idx) accumulation groups: drop the tiny xl@wl term
        groups = [(0, 0), (1, 0), (0, 1)]
    else:
        xt = nc.dram_tensor("xt", [P, NCHUNK, KSUB, NB], f32, kind="ExternalInput").ap()
        wt = nc.dram_tensor("wt", [P, KSUB, N], f32, kind="ExternalInput").ap()
        x_drams, w_drams, mm_dt = [xt], [wt], f32
        groups = [(0, 0)]

    with tile.TileContext(nc) as tc, ExitStack() as ctx:
        const = ctx.enter_context(tc.tile_pool(name="const", bufs=1))
        xpool = ctx.enter_context(tc.tile_pool(name="x", bufs=2))
        opool = ctx.enter_context(tc.tile_pool(name="o", bufs=4))
        psum = ctx.enter_context(tc.tile_pool(name="psum", bufs=4, space="PSUM"))

        w_sbs = []
        for i, wd in enumerate(w_drams):
            w_sb = const.tile([P, KSUB, N], mm_dt, tag=f"w{i}")
            nc.sync.dma_start(w_sb[:], wd[:])
            w_sbs.append(w_sb)
        bias_sb = const.tile([N, 1], f32)
        nc.sync.dma_start(bias_sb[:], bias[:, None])

        for c in range(NCHUNK):
            x_sbs = []
            for i, xd in enumerate(x_drams):
                x_sb = xpool.tile([P, KSUB, NB], mm_dt, tag=f"x{i}")
                nc.sync.dma_start(x_sb[:], xd[:, c])
                x_sbs.append(x_sb)
            ps = psum.tile([N, NB], f32)
            n_mms = len(groups) * KSUB
            i_mm = 0
            for xi, wi in groups:
                for t in range(KSUB):
                    nc.tensor.matmul(
                        ps[:],
                        lhsT=w_sbs[wi][:, t],
                        rhs=x_sbs[xi][:, t],
                        start=(i_mm == 0),
                        stop=(i_mm == n_mms - 1),
                    )
                    i_mm += 1
            o_sb = opool.tile([N, NB], f32)
            nc.scalar.activation(
                o_sb[:], ps[:], mybir.ActivationFunctionType.Identity,
                bias=bias_sb[:],
            )
            nc.sync.dma_start(outT[:, c * NB : (c + 1) * NB], o_sb[:])

    nc.compile()
    _compiled[mm_mode] = nc
    return nc


def _tile_xt(shard):
    """[SHARD, D] fp32 -> [P, NCHUNK, KSUB, NB]: d = t*128 + p, b = c*512 + j."""
    # shard.T is [D, SHARD]; reshape D -> (t, p), SHARD -> (c, j); put p first.
    return np.ascontiguousarray(
        shard.T.reshape(KSUB, P, NCHUNK, NB).transpose(1, 2, 0, 3)
    )


def _tile_w(w):
    """[D, N] -> [P, KSUB, N]."""
    return np.ascontiguousarray(w.reshape(KSUB, P, N).transpose(1, 0, 2))


def _host_prep(x, s, a, b):
    sa = s[None, :, :].astype(np.float32) * a.astype(np.float32)
    w = np.einsum("rij,rkl->ikjl", sa, b.astype(np.float32))
    w = np.ascontiguousarray(w.reshape(D, N), dtype=np.float32)

    in_maps = []
    if MM_MODE == "bf16x3":
        wh32 = w.astype(ml_dtypes.bfloat16).astype(np.float32)
        wh = _tile_w(wh32).astype(ml_dtypes.bfloat16)
        wl = _tile_w(w - wh32).astype(ml_dtypes.bfloat16)
        for i in range(N_CORES):
            xt = _tile_xt(x[i * SHARD : (i + 1) * SHARD])
            xh32 = xt.astype(ml_dtypes.bfloat16).astype(np.float32)
            xh = xh32.astype(ml_dtypes.bfloat16)
            xl = (xt - xh32).astype(ml_dtypes.bfloat16)
            in_maps.append({"xh": xh, "xl": xl, "wh": wh, "wl": wl})
    else:
        wt = _tile_w(w)
        for i in range(N_CORES):
            xt = _tile_xt(x[i * SHARD : (i + 1) * SHARD])
            in_maps.append({"xt": xt, "wt": wt})
    return in_maps


def kernel(x, s, a, b, bias, _trace=False):
    in_maps = _host_prep(
        np.asarray(x, dtype=np.float32),
        np.asarray(s, dtype=np.float32),
        np.asarray(a, dtype=np.float32),
        np.asarray(b, dtype=np.float32),
    )
    bias = np.ascontiguousarray(np.asarray(bias, dtype=np.float32))
    for m in in_maps:
        m["bias"] = bias
    nc = _build(MM_MODE)
    res = run_bass_kernel_spmd(nc, in_maps, list(range(N_CORES)), trace=_trace)
    out = np.concatenate(
        [np.asarray(r["outT"]).T for r in res.results], axis=0
    ).astype(np.float32)
    if _trace:
        return out, res
    return out


# revision 6
# speedup vs baseline: 1.0683x; 1.0081x over previous
"""BlockwiseKronLinear forward on 8 trn2 NeuronCores.

Math: w = reshape(einsum('rij,rkl->ikjl', s*a, b), (4096, 64));
      out = x @ w + bias    with x (32768, 4096) fp32.

Strategy (data-parallel along batch, per the sharding hint):
  - Host: build the tiny w (1 MB) from the Kron factors; shard x along
    batch into 8 x 4096 rows; lay each core's shard out TRANSPOSED and
    tiled ([p=128, chunk=8, ksub=32, nb=512], d = ksub*128 + p,
    batch = chunk*512 + nb) so the contraction dim lands on SBUF
    partitions (fp32 has no DMA-transpose on trn2) and every chunk is
    one fully-contiguous-per-partition 8 MB DMA.
  - Device (identical SPMD program per core): stream the chunks in,
    accumulate outT = w.T @ xT in PSUM over the contraction subtiles
    (w stationary, x the 512-wide moving operand), fuse the bias add on
    ScalarE, write outT [64, 4096] back.
  - Host: gather, transpose back to [32768, 64].

Matmul dtype modes (KRON_MM_MODE):
  - 'bf16x3' (default): split x and w into bf16 hi+lo on host and
    accumulate xh@wh + xl@wh + xh@wl in fp32 PSUM. 3 PE passes at
    1 cyc/row instead of fp32's 4 cyc/row; rel err ~1e-5.
  - 'fp32': exact fp32 matmul (PE runs it as 2x2 half-speed passes).

The kernel is memory-bound: 64 MB of x per core vs ~358 GB/s HBM
=> ~178 us floor.
"""

import os
import sys

for _p in ("/opt/trn_rl_repo", "/root/.axon_site/_ro/trn_rl_repo"):
    if os.path.isdir(_p) and _p not in sys.path:
        sys.path.append(_p)

import numpy as np
import ml_dtypes
from contextlib import ExitStack

import concourse.bass as bass
import concourse.tile as tile
from concourse import bacc, mybir
from concourse.bass_utils import run_bass_kernel_spmd

N_CORES = 8
BATCH, D, N = 32768, 4096, 64
SHARD = BATCH // N_CORES          # 4096 batch rows per core
P = 128                           # SBUF partitions
KSUB = D // P                     # 32 contraction subtiles
NB = 512                          # moving (batch) columns per matmul
NCHUNK = SHARD // NB              # 8 chunks per core

MM_MODE = os.environ.get("KRON_MM_MODE", "bf16x3")

_compiled = {}


def _build(mm_mode: str):
    if mm_mode in _compiled:
        return _compiled[mm_mode]

    nc = bacc.Bacc(
        "TRN2",
        target_bir_lowering=False,
        debug=False,
        num_devices=N_CORES,
    )
    f32 = mybir.dt.float32
    bf16 = mybir.dt.bfloat16

    bias = nc.dram_tensor("bias", [N], f32, kind="ExternalInput").ap()
    outT = nc.dram_tensor("outT", [N, SHARD], f32, kind="ExternalOutput").ap()

    if mm_mode == "bf16x3":
        # (x dram tensor, w dram tensor) per accumulation group
        xh = nc.dram_tensor("xh", [P, NCHUNK, KSUB, NB], bf16, kind="ExternalInput").ap()
        xl = nc.dram_tensor("xl", [P, NCHUNK, KSUB, NB], bf16, kind="ExternalInput").ap()
        wh = nc.dram_tensor("wh", [P, KSUB, N], bf16, kind="ExternalInput").ap()
        wl = nc.dram_tensor("wl", [P, KSUB, N], bf16, kind="ExternalInput").ap()
        x_drams, w_drams, mm_dt = [xh, xl], [wh, wl], bf16
        # (x_idx, w_idx) accumulation groups: drop the tiny xl@wl term
        groups = [(0, 0), (1, 0), (0, 1)]
    else:
        xt = nc.dram_tensor("xt", [P, NCHUNK, KSUB, NB], f32, kind="ExternalInput").ap()
        wt = nc.dram_tensor("wt", [P, KSUB, N], f32, kind="ExternalInput").ap()
        x_drams, w_drams, mm_dt = [xt], [wt], f32
        groups = [(0, 0)]

    with tile.TileContext(nc) as tc, ExitStack() as ctx:
        const = ctx.enter_context(tc.tile_pool(name="const", bufs=1))
        xpool = ctx.enter_context(tc.tile_pool(name="x", bufs=2))
        opool = ctx.enter_context(tc.tile_pool(name="o", bufs=4))
        psum = ctx.enter_context(tc.tile_pool(name="psum", bufs=4, space="PSUM"))

        w_sbs = []
        for i, wd in enumerate(w_drams):
            w_sb = const.tile([P, KSUB, N], mm_dt, tag=f"w{i}")
            nc.sync.dma_start(w_sb[:], wd[:])
            w_sbs.append(w_sb)
        bias_sb = const.tile([N, 1], f32)
        nc.sync.dma_start(bias_sb[:], bias[:, None])

        TG = 8                      # ksub per DMA piece
        NG = KSUB // TG             # pieces per (tensor, chunk)
        for c in range(NCHUNK):
            # x_sbs[tensor_idx][group] -> [P, TG, NB] tile
            x_sbs = [[None] * NG for _ in x_drams]
            for i, xd in enumerate(x_drams):
                for g in range(NG):
                    x_sb = xpool.tile([P, TG, NB], mm_dt, tag=f"x{i}g{g}")
                    nc.sync.dma_start(x_sb[:], xd[:, c, g * TG : (g + 1) * TG])
                    x_sbs[i][g] = x_sb
            ps = psum.tile([N, NB], f32)
            n_mms = len(groups) * KSUB
            i_mm = 0
            for xi, wi in groups:
                for t in range(KSUB):
                    nc.tensor.matmul(
                        ps[:],
                        lhsT=w_sbs[wi][:, t],
                        rhs=x_sbs[xi][t // TG][:, t % TG],
                        start=(i_mm == 0),
                        stop=(i_mm == n_mms - 1),
                    )
                    i_mm += 1
            o_sb = opool.tile([N, NB], f32)
            nc.scalar.activation(
                o_sb[:], ps[:], mybir.ActivationFunctionType.Identity,
                bias=bias_sb[:],
            )
            nc.sync.dma_start(outT[:, c * NB : (c + 1) * NB], o_sb[:])

    nc.compile()
    _compiled[mm_mode] = nc
    return nc


def _tile_xt(shard):
    """[SHARD, D] fp32 -> [P, NCHUNK, KSUB, NB]: d = t*128 + p, b = c*512 + j."""
    # shard.T is [D, SHARD]; reshape D -> (t, p), SHARD -> (c, j); put p first.
    return np.ascontiguousarray(
        shard.T.reshape(KSUB, P, NCHUNK, NB).transpose(1, 2, 0, 3)
    )


def _tile_w(w):
    """[D, N] -> [P, KSUB, N]."""
    return np.ascontiguousarray(w.reshape(KSUB, P, N).transpose(1, 0, 2))


def _host_prep(x, s, a, b):
    sa = s[None, :, :].astype(np.float32) * a.astype(np.float32)
    w = np.einsum("rij,rkl->ikjl", sa, b.astype(np.float32))
    w = np.ascontiguousarray(w.reshape(D, N), dtype=np.float32)

    in_maps = []
    if MM_MODE == "bf16x3":
        wh32 = w.astype(ml_dtypes.bfloat16).astype(np.float32)
        wh = _tile_w(wh32).astype(ml_dtypes.bfloat16)
        wl = _tile_w(w - wh32).astype(ml_dtypes.bfloat16)
        for i in range(N_CORES):
            xt = _tile_xt(x[i * SHARD : (i + 1) * SHARD])
            xh32 = xt.astype(ml_dtypes.bfloat16).astype(np.float32)
            xh = xh32.astype(ml_dtypes.bfloat16)
            xl = (xt - xh32).astype(ml_dtypes.bfloat16)
            in_maps.append({"xh": xh, "xl": xl, "wh": wh, "wl": wl})
    else:
        wt = _tile_w(w)
        for i in range(N_CORES):
            xt = _tile_xt(x[i * SHARD : (i + 1) * SHARD])
            in_maps.append({"xt": xt, "wt": wt})
    return in_maps


def kernel(x, s, a, b, bias, _trace=False):
    in_maps = _host_prep(
        np.asarray(x, dtype=np.float32),
        np.asarray(s, dtype=np.float32),
        np.asarray(a, dtype=np.float32),
        np.asarray(b, dtype=np.float32),
    )
    bias = np.ascontiguousarray(np.asarray(bias, dtype=np.float32))
    for m in in_maps:
        m["bias"] = bias
    nc = _build(MM_MODE)
    res = run_bass_kernel_spmd(nc, in_maps, list(range(N_CORES)), trace=_trace)
    out = np.concatenate(
        [np.asarray(r["outT"]).T for r in res.results], axis=0
    ).astype(np.float32)
    if _trace:
        return out, res
    return out
